# revision 15
# baseline (speedup 1.0000x reference)
"""Trainium2 Bass kernel for nn_BatchedMonomialFactor.

Math (per batch row b):
  logits = (x @ W_perm).reshape(R, B, B) / TAU
  soft   = sinkhorn_5(logits)            (5x row/col normalize, exp space)
  idx    = argmax_i soft[r, i, j]  -> hard one-hot over i
  h_perm[r, i] = sum_j [i == idx[r, j]] * h[r, j]
  out[r, i] = sigmoid(x@W_alpha)[r,i] * tanh(x@W_diag)[r,i] * h_perm[r,i]

Sharding: model-parallel over R (64 r-blocks -> 8 per core); every core
reads the full x_t, weights/h/out are sliced by r; no communication.
The forward output uses only the HARD permutation (straight-through),
and a positive per-column scale cannot change a column argmax, so the
final col-normalize of sinkhorn is skipped.

Engine split (pairs of 128-row batch tiles are fused into single ops
to halve Vector-engine instruction overhead): PE does the three matmuls
(fp32 for exact argmax fidelity); ACT does exp-eviction straight out of
PSUM (fused exp(2z)) plus the exps of the sigmoid/tanh path, which is
rewritten in exps so only one ACT table set is ever loaded; DVE does
the sinkhorn reduces/scales (its ~800us of 1x fp32 passes is the
critical path; GpSimd offload was tried and crashes this environment's
runtime, and no fused multiply+segmented-reduce op exists).
"""

from contextlib import ExitStack

import numpy as np

import concourse.bass as bass
import concourse.tile as tile
from concourse import bacc, mybir
from concourse.bass_utils import run_bass_kernel_spmd

N_CORES = 8
BATCH = 2048
D = 1024
R = 64
B = 16
TAU = 0.5
ITERS = 5

RG = R // N_CORES           # r-blocks per core = 8
NCOL = RG * B * B           # perm-logit cols per core = 2048
DCOL = RG * B               # diag/alpha cols per core = 128
P = 128                     # partitions
NT = BATCH // P             # batch tiles = 16
KT = D // P                 # contraction tiles = 8
F32 = mybir.dt.float32
AF = mybir.ActivationFunctionType
OP = mybir.AluOpType

# The ACT table-set chooser maps Exp -> exp_and_others and Ln ->
# natural_log (first set containing each func), which thrashes a ~2.7us
# table load on every exp<->ln switch. Our kernel only uses Exp and Ln;
# make natural_log_exp_and_others (which has both) the only candidate.
# Set ids are positional, so the dict keeps its original order/size.
import concourse.bacc as _bacc_mod
from concourse import hw_specs as _hw_specs

_orig_get_act_tables = _hw_specs.get_activation_tables


def _patched_get_act_tables(module_arch):
    tabs = _orig_get_act_tables(module_arch)
    return {
        name: (funcs if name == "natural_log_exp_and_others"
               else funcs - {AF.Exp, AF.Ln})
        for name, funcs in tabs.items()
    }


_bacc_mod.get_activation_tables = _patched_get_act_tables


def _build(reps=1, ablate=(), kbufs=3, sbufs=3, recip_eng='dve', tpg=2, xbufs=3,
           mm_f32r=False, iters=ITERS, kt_rs=False):
    ablate = set(ablate)
    F32R = mybir.dt.float32r
    MMDT = F32R if mm_f32r else F32

    def mmcast(ap):
        return ap

    def dmacast(ap):
        return ap.bitcast(F32R) if mm_f32r else ap
    nc = bacc.Bacc("TRN2", target_bir_lowering=False, debug=False,
                   num_devices=N_CORES)
    xT = nc.dram_tensor("xT", [D, BATCH], F32, kind="ExternalInput")
    wp = nc.dram_tensor("wp", [D, NCOL], F32, kind="ExternalInput")
    wda = nc.dram_tensor("wda", [D, 2 * DCOL], F32, kind="ExternalInput")
    hs = nc.dram_tensor("hs", [BATCH, DCOL], F32, kind="ExternalInput")
    out = nc.dram_tensor("out", [BATCH, DCOL], F32, kind="ExternalOutput")

    with tile.TileContext(nc) as tc, ExitStack() as ctx:
        singles = ctx.enter_context(tc.tile_pool(name="singles", bufs=1))
        kpool = ctx.enter_context(tc.tile_pool(name="kpool", bufs=kbufs))
        small = ctx.enter_context(tc.tile_pool(name="small", bufs=sbufs))
        pspool = ctx.enter_context(tc.tile_pool(name="ps", bufs=2, space="PSUM"))
        if kt_rs:
            # psd moves to its own single buffer so psT/psRS fit in PSUM:
            # psK 2x2 + psd 1x2 + psT 1x1 + psRS 1x1 = 8 banks.
            psdpool = ctx.enter_context(
                tc.tile_pool(name="psd", bufs=1, space="PSUM"))
            pstpool = ctx.enter_context(
                tc.tile_pool(name="pst", bufs=1, space="PSUM"))
            psrpool = ctx.enter_context(
                tc.tile_pool(name="psr", bufs=1, space="PSUM"))
            ktpool = ctx.enter_context(tc.tile_pool(name="ktpool", bufs=2))
        else:
            psdpool = pspool

        # Resident operands: W_perm slice, [W_diag | W_alpha] slice.
        # Load the first 512-column chunk of every k first so the first
        # tile's matmuls can start while the rest streams in.
        wps, wdas = [], []
        for k in range(KT):
            w = singles.tile([P, NCOL], MMDT, tag=f"wp{k}")
            wps.append(w)
            w2 = singles.tile([P, 2 * DCOL], MMDT, tag=f"wda{k}")
            wdas.append(w2)
        for k in range(KT):
            nc.sync.dma_start(out=wps[k][:, 0:512],
                              in_=dmacast(wp.ap()[k * P:(k + 1) * P, 0:512]))
        # bulk weight streaming rides a different DMA queue (ScalarE's)
        # so the first tile's x/h loads on SyncE's queue aren't stuck
        # behind it.
        for k in range(KT):
            nc.scalar.dma_start(out=wdas[k][:],
                                in_=dmacast(wda.ap()[k * P:(k + 1) * P, :]))
        for k in range(KT):
            nc.scalar.dma_start(out=wps[k][:, 512:NCOL],
                                in_=dmacast(wp.ap()[k * P:(k + 1) * P, 512:NCOL]))
        xpool = ctx.enter_context(tc.tile_pool(name="xpool", bufs=xbufs))

        if kt_rs:
            # identity for PE transposes; maskI8[p, m] = 1 iff p//16 == m,
            # turning a transposed K block [(i%8, j) x batch] into the 8
            # per-(g,i) row sums via one matmul per block.
            ident = singles.tile([P, P], F32, tag="ident")
            nc.vector.memset(ident[:], 0.0)
            for p_ in range(P):
                nc.vector.memset(ident[p_:p_ + 1, p_:p_ + 1], 1.0)
            maskI8 = singles.tile([P, 8], F32, tag="maskI8")
            nc.vector.memset(maskI8[:], 0.0)
            for m in range(8):
                nc.vector.memset(maskI8[m * 16:(m + 1) * 16, m:m + 1], 1.0)

        def act_recip(dst, src):
            if recip_eng == 'dve':
                nc.vector.reciprocal(out=dst, in_=src)
                return
            # 1/x = exp(-ln x); ln+exp share one ACT table set.
            n = src.shape[-1] if hasattr(src, 'shape') else DCe
            tmp = small.tile([P, n], F32, tag="lntmp")
            nc.scalar.activation(out=tmp, in_=src, func=AF.Ln)
            nc.scalar.activation(out=dst, in_=tmp, func=AF.Exp, scale=-1.0)

        RGe = RG * tpg          # merged r-groups across tpg batch subtiles
        DCe = DCOL * tpg
        for bt in range((NT // tpg) * reps):
            bt = bt % (NT // tpg)

            # per-subtile x^T slices, streamed
            xts = []
            for s_ in range(tpg):
                xsub = []
                for k in range(KT):
                    xt = xpool.tile([P, P], MMDT, tag=f"xt{k}_{s_}")
                    nc.sync.dma_start(
                        out=xt,
                        in_=dmacast(
                            xT.ap()[k * P:(k + 1) * P,
                                    (bt * tpg + s_) * P:(bt * tpg + s_ + 1) * P]))
                    xsub.append(xt)
                xts.append(xsub)

            K_t = kpool.tile([P, RGe, B, B], F32, tag="K")
            Kflat = K_t[:].rearrange("p g i j -> p (g i j)")

            # logits matmul in halves of 1024 (2 PSUM banks each);
            # evict through ACT with fused exp(2*z)  [1/TAU = 2].
            for s_ in range(tpg):
                for half in range(2):
                    ps = pspool.tile([P, 1024], F32, tag="psK")
                    for nb in range(2):
                        ncol0 = half * 1024 + nb * 512
                        for k in range(KT):
                            nc.tensor.matmul(
                                out=ps[:, nb * 512:(nb + 1) * 512],
                                lhsT=mmcast(xts[s_][k][:]),
                                rhs=mmcast(wps[k][:, ncol0:ncol0 + 512]),
                                start=(k == 0),
                                stop=(k == KT - 1),
                            )
                    nc.scalar.activation(
                            out=Kflat[:, (s_ * 2 + half) * 1024:
                                      (s_ * 2 + half + 1) * 1024],
                            in_=ps[:],
                            func=AF.Exp,
                            scale=2.0,
                        )

            # K^T via PE transpose, then per-block mask matmuls give the
            # iteration-1 row sums in PSUM without a DVE reduce pass.
            psRS = None
            if kt_rs:
                nblk = RGe * 2
                ktb = ktpool.tile([P, nblk * P], F32, tag="ktb")
                for w in range(nblk // 4):
                    psT = pstpool.tile([P, 512], F32, tag="psT")
                    for q in range(4):
                        c = w * 4 + q
                        nc.tensor.transpose(
                            out=psT[:, q * P:(q + 1) * P],
                            in_=Kflat[:, c * P:(c + 1) * P],
                            identity=ident[:])
                    nc.scalar.copy(out=ktb[:, w * 512:(w + 1) * 512],
                                   in_=psT[:])
                psRS = psrpool.tile([P, RGe * B], F32, tag="psRS")
                for c in range(nblk):
                    g, h = c // 2, c % 2
                    nc.tensor.matmul(
                        out=psRS[:, g * B + 8 * h:g * B + 8 * h + 8],
                        lhsT=ktb[:, c * P:(c + 1) * P],
                        rhs=maskI8[:],
                        start=True, stop=True)

            # diag/alpha matmul: [x @ Wd | x @ Wa] -> one PSUM bank.
            psd = psdpool.tile([P, tpg * 2 * DCOL], F32, tag="psD")
            for s_ in range(tpg):
                for k in range(KT):
                    nc.tensor.matmul(
                        out=psd[:, s_ * 2 * DCOL:(s_ + 1) * 2 * DCOL],
                        lhsT=mmcast(xts[s_][k][:]),
                        rhs=mmcast(wdas[k][:]),
                        start=(k == 0),
                        stop=(k == KT - 1),
                    )
            # sigmoid(a)*tanh(d) = (e2d - 1) / ((1 + e2d) * (1 + ena))
            e2d = small.tile([P, DCe], F32, tag="e2d")
            ena = small.tile([P, DCe], F32, tag="ena")
            for s_ in range(tpg):
                nc.scalar.activation(
                    out=e2d[:, s_ * DCOL:(s_ + 1) * DCOL],
                    in_=psd[:, s_ * 2 * DCOL:s_ * 2 * DCOL + DCOL],
                    func=AF.Exp, scale=2.0)
                nc.scalar.activation(
                    out=ena[:, s_ * DCOL:(s_ + 1) * DCOL],
                    in_=psd[:, s_ * 2 * DCOL + DCOL:(s_ + 1) * 2 * DCOL],
                    func=AF.Exp, scale=-1.0)
            num = small.tile([P, DCe], F32, tag="num")
            nc.vector.tensor_scalar_sub(out=num, in0=e2d, scalar1=1.0)
            den = small.tile([P, DCe], F32, tag="den")
            nc.vector.scalar_tensor_tensor(out=den, in0=e2d, scalar=1.0,
                                           in1=ena, op0=OP.add, op1=OP.mult)
            dpa = small.tile([P, DCe], F32, tag="dpa")
            # denom = (1+e2d)*(1+ena) = (e2d+1) + (e2d+1)*ena
            nc.vector.scalar_tensor_tensor(out=dpa, in0=e2d, scalar=1.0,
                                           in1=den, op0=OP.add, op1=OP.add)
            rden = small.tile([P, DCe], F32, tag="rden")
            act_recip(rden, dpa)
            dv = small.tile([P, DCe], F32, tag="dv")
            nc.vector.tensor_mul(out=dv, in0=num, in1=rden)

            def sinkhorn_final(g0, ng):
                # sinkhorn + hard-permutation + output for r-groups
                # [g0, g0+ng) of this tile's merged K. Splitting the first
                # tile into halves lets DVE start before all evictions land.
                Xs = K_t[:, g0:g0 + ng]                 # [P, ng, i, j]
                Xti = Xs.transpose([0, 1, 3, 2])        # [P, ng, j, i]
                DCs = ng * B
                csl = slice(g0 * B, (g0 + ng) * B)

                def bcast_gi(t):   # (g,i)-indexed -> broadcast over j
                    return (t[:].rearrange("p (g i) -> p g i", g=ng)
                            .unsqueeze(3).to_broadcast([P, ng, B, B]))

                def bcast_gj(t):   # (g,j)-indexed -> broadcast over i
                    return (t[:].rearrange("p (g j) -> p g j", g=ng)
                            .unsqueeze(2).to_broadcast([P, ng, B, B]))

                for it in range(iters):
                    if it == 0 and psRS is not None:
                        rs = psRS[:, csl]
                    else:
                        rs = small.tile([P, DCs], F32, tag="rs")
                        nc.vector.reduce_sum(out=rs, in_=Xs,
                                             axis=mybir.AxisListType.X)
                    rr = small.tile([P, DCs], F32, tag="rr")
                    act_recip(rr, rs)
                    nc.vector.tensor_tensor(out=Xs, in0=Xs, in1=bcast_gi(rr),
                                            op=OP.mult)
                    if it < iters - 1:
                        cs = small.tile([P, DCs], F32, tag="cs")
                        nc.vector.reduce_sum(out=cs, in_=Xti,
                                             axis=mybir.AxisListType.X)
                        rc = small.tile([P, DCs], F32, tag="rc")
                        act_recip(rc, cs)
                        nc.vector.tensor_tensor(out=Xs, in0=Xs,
                                                in1=bcast_gj(rc), op=OP.mult)

                # column max over i -> hard assignment mask -> h gather.
                M = small.tile([P, DCs], F32, tag="M")
                nc.vector.reduce_max(out=M, in_=Xti, axis=mybir.AxisListType.X)
                nc.vector.tensor_tensor(out=Xs, in0=Xs, in1=bcast_gj(M),
                                        op=OP.is_equal)
                nc.vector.tensor_tensor(out=Xs, in0=Xs,
                                        in1=bcast_gj(h_t[:, csl]), op=OP.mult)
                hp = small.tile([P, DCs], F32, tag="hp")
                nc.vector.reduce_sum(out=hp, in_=Xs, axis=mybir.AxisListType.X)
                nc.vector.tensor_mul(out=o_t[:, csl], in0=hp, in1=dv[:, csl])

            h_t = small.tile([P, DCe], F32, tag="h")
            for s_ in range(tpg):
                b0 = (bt * tpg + s_) * P
                nc.sync.dma_start(out=h_t[:, s_ * DCOL:(s_ + 1) * DCOL],
                                  in_=hs.ap()[b0:b0 + P, :])
            o_t = small.tile([P, DCe], F32, tag="o")

            if bt == 0:
                q = RGe // (2 * tpg)   # one eviction's worth of r-groups
                for s_ in range(2 * tpg):
                    sinkhorn_final(s_ * q, q)
            else:
                sinkhorn_final(0, RGe)

            for s_ in range(tpg):
                b0 = (bt * tpg + s_) * P
                nc.sync.dma_start(out=out.ap()[b0:b0 + P, :],
                                  in_=o_t[:, s_ * DCOL:(s_ + 1) * DCOL])

    nc.compile()
    return nc


_NC = None


def _get_nc():
    global _NC
    if _NC is None:
        _NC = _build()
    return _NC


def kernel(x_t, h, W_perm, W_diag, W_alpha):
    x_t = np.ascontiguousarray(np.asarray(x_t, dtype=np.float32))
    h = np.asarray(h, dtype=np.float32)
    W_perm = np.asarray(W_perm, dtype=np.float32)
    W_diag = np.asarray(W_diag, dtype=np.float32)
    W_alpha = np.asarray(W_alpha, dtype=np.float32)

    xT = np.ascontiguousarray(x_t.T)                          # [D, BATCH]
    wp4 = W_perm.reshape(D, R, B * B)
    wd3 = W_diag.reshape(D, R, B)
    wa3 = W_alpha.reshape(D, R, B)
    h3 = h.reshape(BATCH, R, B)

    in_maps = []
    for c in range(N_CORES):
        rsl = slice(c * RG, (c + 1) * RG)
        in_maps.append({
            "xT": xT,
            "wp": np.ascontiguousarray(wp4[:, rsl].reshape(D, NCOL)),
            "wda": np.ascontiguousarray(
                np.concatenate([wd3[:, rsl].reshape(D, DCOL),
                                wa3[:, rsl].reshape(D, DCOL)], axis=1)),
            "hs": np.ascontiguousarray(h3[:, rsl].reshape(BATCH, DCOL)),
        })

    global _last_in_maps
    _last_in_maps = in_maps
    res = run_bass_kernel_spmd(_get_nc(), in_maps, core_ids=list(range(N_CORES)))
    parts = [res.results[c]["out"].reshape(BATCH, RG, B) for c in range(N_CORES)]
    return np.concatenate(parts, axis=1).reshape(BATCH, R * B).astype(np.float32)



# revision 35
# speedup vs baseline: 1.2376x; 1.2376x over previous
"""Trainium2 Bass kernel for nn_BatchedMonomialFactor.

Math (per batch row b):
  logits = (x @ W_perm).reshape(R, B, B) / TAU
  soft   = sinkhorn_5(logits)            (5x row/col normalize, exp space)
  idx    = argmax_i soft[r, i, j]  -> hard one-hot over i
  h_perm[r, i] = sum_j [i == idx[r, j]] * h[r, j]
  out[r, i] = sigmoid(x@W_alpha)[r,i] * tanh(x@W_diag)[r,i] * h_perm[r,i]

Sharding: model-parallel over R (64 r-blocks -> 8 per core); every core
reads the full x_t, weights/h/out are sliced by r; no communication.
The forward output uses only the HARD permutation (straight-through),
and a positive per-column scale cannot change a column argmax, so the
final col-normalize of sinkhorn is skipped.

Engine split (pairs of 128-row batch tiles are fused into single ops
to halve Vector-engine instruction overhead): PE does the three matmuls
(fp32 for exact argmax fidelity); ACT does exp-eviction straight out of
PSUM (fused exp(2z)) plus the exps of the sigmoid/tanh path, which is
rewritten in exps so only one ACT table set is ever loaded; DVE does
the sinkhorn reduces/scales (its ~800us of 1x fp32 passes is the
critical path; GpSimd offload was tried and crashes this environment's
runtime, and no fused multiply+segmented-reduce op exists).
"""

from contextlib import ExitStack

import numpy as np

import concourse.bass as bass
import concourse.tile as tile
from concourse import bacc, mybir
from concourse.bass_utils import run_bass_kernel_spmd

N_CORES = 8
BATCH = 2048
D = 1024
R = 64
B = 16
TAU = 0.5
ITERS = 5

RG = R // N_CORES           # r-blocks per core = 8
NCOL = RG * B * B           # perm-logit cols per core = 2048
DCOL = RG * B               # diag/alpha cols per core = 128
P = 128                     # partitions
NT = BATCH // P             # batch tiles = 16
KT = D // P                 # contraction tiles = 8
F32 = mybir.dt.float32
AF = mybir.ActivationFunctionType
OP = mybir.AluOpType

# The ACT table-set chooser maps Exp -> exp_and_others and Ln ->
# natural_log (first set containing each func), which thrashes a ~2.7us
# table load on every exp<->ln switch. Our kernel only uses Exp and Ln;
# make natural_log_exp_and_others (which has both) the only candidate.
# Set ids are positional, so the dict keeps its original order/size.
import concourse.bacc as _bacc_mod
from concourse import hw_specs as _hw_specs

_orig_get_act_tables = _hw_specs.get_activation_tables
_ACT_TABLE_MODE = "ln_exp"


def _patched_get_act_tables(module_arch):
    tabs = _orig_get_act_tables(module_arch)
    if _ACT_TABLE_MODE == "exp_tanh":
        # exp_and_others holds both Exp and Tanh: one table set serves the
        # evictions and the sigmoid/tanh path with zero table swaps.
        return {
            name: (funcs if name == "exp_and_others"
                   else funcs - {AF.Exp, AF.Tanh})
            for name, funcs in tabs.items()
        }
    return {
        name: (funcs if name == "natural_log_exp_and_others"
               else funcs - {AF.Exp, AF.Ln})
        for name, funcs in tabs.items()
    }


_bacc_mod.get_activation_tables = _patched_get_act_tables


def _build(reps=1, ablate=(), kbufs=3, sbufs=3, recip_eng='dve', tpg=2, xbufs=3,
           mm_f32r=False, iters=ITERS, kt_rs=False, hp_pe=False, dv_act=False):
    ablate = set(ablate)
    global _ACT_TABLE_MODE
    _ACT_TABLE_MODE = "exp_tanh" if dv_act else "ln_exp"
    F32R = mybir.dt.float32r
    MMDT = F32R if mm_f32r else F32

    def mmcast(ap):
        return ap

    def dmacast(ap):
        return ap.bitcast(F32R) if mm_f32r else ap
    nc = bacc.Bacc("TRN2", target_bir_lowering=False, debug=False,
                   num_devices=N_CORES)
    xT = nc.dram_tensor("xT", [D, BATCH], F32, kind="ExternalInput")
    wp = nc.dram_tensor("wp", [D, NCOL], F32, kind="ExternalInput")
    wda = nc.dram_tensor("wda", [D, 2 * DCOL], F32, kind="ExternalInput")
    hs = nc.dram_tensor("hs", [BATCH, DCOL], F32, kind="ExternalInput")
    out = nc.dram_tensor("out", [BATCH, DCOL], F32, kind="ExternalOutput")

    with tile.TileContext(nc) as tc, ExitStack() as ctx:
        singles = ctx.enter_context(tc.tile_pool(name="singles", bufs=1))
        kpool = ctx.enter_context(tc.tile_pool(name="kpool", bufs=kbufs))
        small = ctx.enter_context(tc.tile_pool(name="small", bufs=sbufs))
        pspool = ctx.enter_context(tc.tile_pool(name="ps", bufs=2, space="PSUM"))
        if kt_rs:
            # psd moves to its own single buffer so psT/psRS fit in PSUM.
            # With hp_pe the perm eviction chunks shrink to 512 so psK only
            # needs 2 banks: psK 2x1 + psd 1x2 + psT/psRS/psT2/psHP 1x1 each.
            psdpool = ctx.enter_context(
                tc.tile_pool(name="psd", bufs=1, space="PSUM"))
            pstpool = ctx.enter_context(
                tc.tile_pool(name="pst", bufs=1, space="PSUM"))
            psrpool = ctx.enter_context(
                tc.tile_pool(name="psr", bufs=1, space="PSUM"))
            ktpool = ctx.enter_context(tc.tile_pool(name="ktpool", bufs=2))
        else:
            psdpool = pspool
        kchunk = 512 if hp_pe else 1024

        # Resident operands: W_perm slice, [W_diag | W_alpha] slice.
        # Load the first 512-column chunk of every k first so the first
        # tile's matmuls can start while the rest streams in.
        wps, wdas = [], []
        for k in range(KT):
            w = singles.tile([P, NCOL], MMDT, tag=f"wp{k}")
            wps.append(w)
            w2 = singles.tile([P, 2 * DCOL], MMDT, tag=f"wda{k}")
            wdas.append(w2)
        for k in range(KT):
            nc.sync.dma_start(out=wps[k][:, 0:512],
                              in_=dmacast(wp.ap()[k * P:(k + 1) * P, 0:512]))
        # bulk weight streaming rides a different DMA queue (ScalarE's)
        # so the first tile's x/h loads on SyncE's queue aren't stuck
        # behind it.
        for k in range(KT):
            nc.scalar.dma_start(out=wdas[k][:],
                                in_=dmacast(wda.ap()[k * P:(k + 1) * P, :]))
        for k in range(KT):
            nc.scalar.dma_start(out=wps[k][:, 512:NCOL],
                                in_=dmacast(wp.ap()[k * P:(k + 1) * P, 512:NCOL]))
        xpool = ctx.enter_context(tc.tile_pool(name="xpool", bufs=xbufs))

        if kt_rs:
            # identity for PE transposes; maskI8[p, m] = 1 iff p//16 == m,
            # turning a transposed K block [(i%8, j) x batch] into the 8
            # per-(g,i) row sums via one matmul per block.
            ident_np = np.eye(P, dtype=np.float32)
            mi8_np = (np.arange(P)[:, None] // 16
                      == np.arange(8)[None, :]).astype(np.float32)
            ident_dram = nc.inline_tensor(ident_np, name="identc")
            mi8_dram = nc.inline_tensor(mi8_np, name="maski8c")
            # hp-variant of the mask; absorbs the 1/2 from the tanh-based
            # sigmoid identity when dv_act is on.
            mh_dram = nc.inline_tensor(
                mi8_np * (0.5 if dv_act else 1.0), name="maskhc")
            ident = singles.tile([P, P], F32, tag="ident")
            nc.sync.dma_start(out=ident[:], in_=ident_dram.ap())
            maskI8 = singles.tile([P, 8], F32, tag="maskI8")
            nc.sync.dma_start(out=maskI8[:], in_=mi8_dram.ap())
            maskH = singles.tile([P, 8], F32, tag="maskH")
            nc.sync.dma_start(out=maskH[:], in_=mh_dram.ap())

        def act_recip(dst, src):
            if recip_eng == 'dve':
                nc.vector.reciprocal(out=dst, in_=src)
                return
            # 1/x = exp(-ln x); ln+exp share one ACT table set.
            n = src.shape[-1] if hasattr(src, 'shape') else DCe
            tmp = small.tile([P, n], F32, tag="lntmp")
            nc.scalar.activation(out=tmp, in_=src, func=AF.Ln)
            nc.scalar.activation(out=dst, in_=tmp, func=AF.Exp, scale=-1.0)

        RGe = RG * tpg          # merged r-groups across tpg batch subtiles
        DCe = DCOL * tpg
        assert not (hp_pe and not kt_rs), "hp_pe requires kt_rs machinery"
        pending = None
        for bt in range((NT // tpg) * reps):
            bt = bt % (NT // tpg)

            # per-subtile x^T slices, streamed
            xts = []
            for s_ in range(tpg):
                xsub = []
                for k in range(KT):
                    xt = xpool.tile([P, P], MMDT, tag=f"xt{k}_{s_}")
                    nc.sync.dma_start(
                        out=xt,
                        in_=dmacast(
                            xT.ap()[k * P:(k + 1) * P,
                                    (bt * tpg + s_) * P:(bt * tpg + s_ + 1) * P]))
                    xsub.append(xt)
                xts.append(xsub)

            K_t = kpool.tile([P, RGe, B, B], F32, tag="K")
            Kflat = K_t[:].rearrange("p g i j -> p (g i j)")

            # logits matmul in PSUM chunks; evict through ACT with fused
            # exp(2*z)  [1/TAU = 2].
            for s_ in range(tpg):
                for ch in range(NCOL // kchunk):
                    ps = pspool.tile([P, kchunk], F32, tag="psK")
                    for nb in range(kchunk // 512):
                        ncol0 = ch * kchunk + nb * 512
                        for k in range(KT):
                            nc.tensor.matmul(
                                out=ps[:, nb * 512:(nb + 1) * 512],
                                lhsT=mmcast(xts[s_][k][:]),
                                rhs=mmcast(wps[k][:, ncol0:ncol0 + 512]),
                                start=(k == 0),
                                stop=(k == KT - 1),
                            )
                    nc.scalar.activation(
                            out=Kflat[:, s_ * NCOL + ch * kchunk:
                                      s_ * NCOL + (ch + 1) * kchunk],
                            in_=ps[:],
                            func=AF.Exp,
                            scale=2.0,
                        )

            # K^T via PE transpose, then per-block mask matmuls give the
            # iteration-1 row sums in PSUM without a DVE reduce pass.
            psRS = None
            if kt_rs:
                nblk = RGe * 2
                psRS = psrpool.tile([P, RGe * B], F32, tag="psRS")
                for w in range(nblk // 4):
                    psT = pstpool.tile([P, 512], F32, tag="psT")
                    for q in range(4):
                        c = w * 4 + q
                        nc.tensor.transpose(
                            out=psT[:, q * P:(q + 1) * P],
                            in_=Kflat[:, c * P:(c + 1) * P],
                            identity=ident[:])
                    ktb = ktpool.tile([P, 512], F32, tag="ktb")
                    nc.scalar.copy(out=ktb[:], in_=psT[:])
                    for q in range(4):
                        c = w * 4 + q
                        g, h = c // 2, c % 2
                        nc.tensor.matmul(
                            out=psRS[:, g * B + 8 * h:g * B + 8 * h + 8],
                            lhsT=ktb[:, q * P:(q + 1) * P],
                            rhs=maskI8[:],
                            start=True, stop=True)

            # diag/alpha matmul: [x @ Wd | x @ Wa] -> one PSUM bank.
            psd = psdpool.tile([P, tpg * 2 * DCOL], F32, tag="psD")
            for s_ in range(tpg):
                for k in range(KT):
                    nc.tensor.matmul(
                        out=psd[:, s_ * 2 * DCOL:(s_ + 1) * 2 * DCOL],
                        lhsT=mmcast(xts[s_][k][:]),
                        rhs=mmcast(wdas[k][:]),
                        start=(k == 0),
                        stop=(k == KT - 1),
                    )
            if dv_act:
                # sigmoid(a)*tanh(d) = (1 + tanh(a/2))*tanh(d) / 2; both
                # tanhs share the exp table set so ACT never swaps tables.
                # dv holds 2x the true value; the 1/2 is folded into the
                # h-mask multiply below.
                t2 = small.tile([P, DCe], F32, tag="e2d")
                t1 = small.tile([P, DCe], F32, tag="ena")
                for s_ in range(tpg):
                    nc.scalar.activation(
                        out=t2[:, s_ * DCOL:(s_ + 1) * DCOL],
                        in_=psd[:, s_ * 2 * DCOL:s_ * 2 * DCOL + DCOL],
                        func=AF.Tanh)
                    nc.scalar.activation(
                        out=t1[:, s_ * DCOL:(s_ + 1) * DCOL],
                        in_=psd[:, s_ * 2 * DCOL + DCOL:(s_ + 1) * 2 * DCOL],
                        func=AF.Tanh, scale=0.5)
                dv = small.tile([P, DCe], F32, tag="dv")
                nc.vector.scalar_tensor_tensor(out=dv, in0=t1, scalar=1.0,
                                               in1=t2, op0=OP.add, op1=OP.mult)
            else:
                # sigmoid(a)*tanh(d) = (e2d - 1) / ((1 + e2d) * (1 + ena))
                e2d = small.tile([P, DCe], F32, tag="e2d")
                ena = small.tile([P, DCe], F32, tag="ena")
                for s_ in range(tpg):
                    nc.scalar.activation(
                        out=e2d[:, s_ * DCOL:(s_ + 1) * DCOL],
                        in_=psd[:, s_ * 2 * DCOL:s_ * 2 * DCOL + DCOL],
                        func=AF.Exp, scale=2.0)
                    nc.scalar.activation(
                        out=ena[:, s_ * DCOL:(s_ + 1) * DCOL],
                        in_=psd[:, s_ * 2 * DCOL + DCOL:(s_ + 1) * 2 * DCOL],
                        func=AF.Exp, scale=-1.0)
                num = small.tile([P, DCe], F32, tag="num")
                nc.vector.tensor_scalar_sub(out=num, in0=e2d, scalar1=1.0)
                den = small.tile([P, DCe], F32, tag="den")
                nc.vector.scalar_tensor_tensor(out=den, in0=e2d, scalar=1.0,
                                               in1=ena, op0=OP.add, op1=OP.mult)
                dpa = small.tile([P, DCe], F32, tag="dpa")
                # denom = (1+e2d)*(1+ena) = (e2d+1) + (e2d+1)*ena
                nc.vector.scalar_tensor_tensor(out=dpa, in0=e2d, scalar=1.0,
                                               in1=den, op0=OP.add, op1=OP.add)
                rden = small.tile([P, DCe], F32, tag="rden")
                act_recip(rden, dpa)
                dv = small.tile([P, DCe], F32, tag="dv")
                nc.vector.tensor_mul(out=dv, in0=num, in1=rden)

            def sinkhorn_final(g0, ng, after_it0=None):
                # sinkhorn + hard-permutation + output for r-groups
                # [g0, g0+ng) of this tile's merged K. Splitting the first
                # tile into halves lets DVE start before all evictions land.
                Xs = K_t[:, g0:g0 + ng]                 # [P, ng, i, j]
                Xti = Xs.transpose([0, 1, 3, 2])        # [P, ng, j, i]
                DCs = ng * B
                csl = slice(g0 * B, (g0 + ng) * B)

                def bcast_gi(t):   # (g,i)-indexed -> broadcast over j
                    return (t[:].rearrange("p (g i) -> p g i", g=ng)
                            .unsqueeze(3).to_broadcast([P, ng, B, B]))

                def bcast_gj(t):   # (g,j)-indexed -> broadcast over i
                    return (t[:].rearrange("p (g j) -> p g j", g=ng)
                            .unsqueeze(2).to_broadcast([P, ng, B, B]))

                for it in range(iters):
                    if it == 0 and psRS is not None:
                        rs = psRS[:, csl]
                    else:
                        rs = small.tile([P, DCs], F32, tag="rs")
                        nc.vector.reduce_sum(out=rs, in_=Xs,
                                             axis=mybir.AxisListType.X)
                    rr = small.tile([P, DCs], F32, tag="rr")
                    act_recip(rr, rs)
                    nc.vector.tensor_tensor(out=Xs, in0=Xs, in1=bcast_gi(rr),
                                            op=OP.mult)
                    if it == 0 and after_it0 is not None:
                        after_it0()
                    if it < iters - 1:
                        cs = small.tile([P, DCs], F32, tag="cs")
                        nc.vector.reduce_sum(out=cs, in_=Xti,
                                             axis=mybir.AxisListType.X)
                        rc = small.tile([P, DCs], F32, tag="rc")
                        act_recip(rc, cs)
                        nc.vector.tensor_tensor(out=Xs, in0=Xs,
                                                in1=bcast_gj(rc), op=OP.mult)

                # column max over i -> hard assignment mask -> h gather.
                if 'argmax' in ablate:
                    nc.vector.tensor_mul(out=o_t[:, csl],
                                         in0=h_t[:, csl], in1=dv[:, csl])
                    return
                M = small.tile([P, DCs], F32, tag="M")
                nc.vector.reduce_max(out=M, in_=Xti, axis=mybir.AxisListType.X)
                nc.vector.tensor_tensor(out=Xs, in0=Xs, in1=bcast_gj(M),
                                        op=OP.is_equal)
                nc.vector.tensor_tensor(out=Xs, in0=Xs,
                                        in1=bcast_gj(h_t[:, csl]), op=OP.mult)
                if hp_pe:
                    # transpose the masked-h product and mask-matmul the
                    # per-(g,i) sums straight into psHP [batch, (g,i)];
                    # the hp*dv multiply is deferred into the next tile.
                    for w in range(2 * ng // 4):
                        psT2 = pstpool.tile([P, 512], F32, tag="psT2")
                        for q in range(4):
                            c = 2 * g0 + w * 4 + q
                            nc.tensor.transpose(
                                out=psT2[:, q * P:(q + 1) * P],
                                in_=Kflat[:, c * P:(c + 1) * P],
                                identity=ident[:])
                        ktb2 = ktpool.tile([P, 512], F32, tag="ktb2")
                        nc.scalar.copy(out=ktb2[:], in_=psT2[:])
                        for q in range(4):
                            c = 2 * g0 + w * 4 + q
                            g, h_ = c // 2, c % 2
                            nc.tensor.matmul(
                                out=psHP[:, g * B + 8 * h_:g * B + 8 * h_ + 8],
                                lhsT=ktb2[:, q * P:(q + 1) * P],
                                rhs=maskH[:],
                                start=True, stop=True)
                    return
                hp = small.tile([P, DCs], F32, tag="hp")
                nc.vector.reduce_sum(out=hp, in_=Xs, axis=mybir.AxisListType.X)
                if dv_act:
                    # dv holds 2x sigmoid*tanh; halve here.
                    nc.vector.scalar_tensor_tensor(
                        out=o_t[:, csl], in0=hp, scalar=0.5, in1=dv[:, csl],
                        op0=OP.mult, op1=OP.mult)
                else:
                    nc.vector.tensor_mul(out=o_t[:, csl], in0=hp,
                                         in1=dv[:, csl])

            h_t = small.tile([P, DCe], F32, tag="h")
            for s_ in range(tpg):
                b0 = (bt * tpg + s_) * P
                nc.sync.dma_start(out=h_t[:, s_ * DCOL:(s_ + 1) * DCOL],
                                  in_=hs.ap()[b0:b0 + P, :])
            o_t = small.tile([P, DCe], F32, tag="o")
            psHP = None
            if hp_pe:
                psHP = psrpool.tile([P, RGe * B], F32, tag="psHP")

            flush_prev = None
            if pending is not None:
                o_p, dv_p, psHP_p, bt_p = pending

                def flush_prev(o_p=o_p, dv_p=dv_p, psHP_p=psHP_p, bt_p=bt_p):
                    nc.vector.tensor_mul(out=o_p[:], in0=psHP_p[:], in1=dv_p[:])
                    for s_ in range(tpg):
                        b0 = (bt_p * tpg + s_) * P
                        nc.sync.dma_start(
                            out=out.ap()[b0:b0 + P, :],
                            in_=o_p[:, s_ * DCOL:(s_ + 1) * DCOL])
                pending = None

            if bt == 0:
                q = RGe // (2 * tpg)   # one eviction's worth of r-groups
                for s_ in range(2 * tpg):
                    sinkhorn_final(s_ * q, q,
                                   after_it0=flush_prev if s_ == 0 else None)
            else:
                sinkhorn_final(0, RGe, after_it0=flush_prev)

            if hp_pe:
                pending = (o_t, dv, psHP, bt)
            else:
                for s_ in range(tpg):
                    b0 = (bt * tpg + s_) * P
                    nc.sync.dma_start(out=out.ap()[b0:b0 + P, :],
                                      in_=o_t[:, s_ * DCOL:(s_ + 1) * DCOL])
        if pending is not None:
            o_p, dv_p, psHP_p, bt_p = pending
            nc.vector.tensor_mul(out=o_p[:], in0=psHP_p[:], in1=dv_p[:])
            for s_ in range(tpg):
                b0 = (bt_p * tpg + s_) * P
                nc.sync.dma_start(out=out.ap()[b0:b0 + P, :],
                                  in_=o_p[:, s_ * DCOL:(s_ + 1) * DCOL])

    nc.compile()
    return nc


_NC = None


def _get_nc():
    global _NC
    if _NC is None:
        _NC = _build()
    return _NC


def kernel(x_t, h, W_perm, W_diag, W_alpha):
    x_t = np.ascontiguousarray(np.asarray(x_t, dtype=np.float32))
    h = np.asarray(h, dtype=np.float32)
    W_perm = np.asarray(W_perm, dtype=np.float32)
    W_diag = np.asarray(W_diag, dtype=np.float32)
    W_alpha = np.asarray(W_alpha, dtype=np.float32)

    xT = np.ascontiguousarray(x_t.T)                          # [D, BATCH]
    wp4 = W_perm.reshape(D, R, B * B)
    wd3 = W_diag.reshape(D, R, B)
    wa3 = W_alpha.reshape(D, R, B)
    h3 = h.reshape(BATCH, R, B)

    in_maps = []
    for c in range(N_CORES):
        rsl = slice(c * RG, (c + 1) * RG)
        in_maps.append({
            "xT": xT,
            "wp": np.ascontiguousarray(wp4[:, rsl].reshape(D, NCOL)),
            "wda": np.ascontiguousarray(
                np.concatenate([wd3[:, rsl].reshape(D, DCOL),
                                wa3[:, rsl].reshape(D, DCOL)], axis=1)),
            "hs": np.ascontiguousarray(h3[:, rsl].reshape(BATCH, DCOL)),
        })

    global _last_in_maps
    _last_in_maps = in_maps
    res = run_bass_kernel_spmd(_get_nc(), in_maps, core_ids=list(range(N_CORES)))
    parts = [res.results[c]["out"].reshape(BATCH, RG, B) for c in range(N_CORES)]
    return np.concatenate(parts, axis=1).reshape(BATCH, R * B).astype(np.float32)



# revision 36
# speedup vs baseline: 1.9679x; 1.5900x over previous
"""Trainium2 Bass kernel for nn_BatchedMonomialFactor.

Math (per batch row b):
  logits = (x @ W_perm).reshape(R, B, B) / TAU
  soft   = sinkhorn_5(logits)            (5x row/col normalize, exp space)
  idx    = argmax_i soft[r, i, j]  -> hard one-hot over i
  h_perm[r, i] = sum_j [i == idx[r, j]] * h[r, j]
  out[r, i] = sigmoid(x@W_alpha)[r,i] * tanh(x@W_diag)[r,i] * h_perm[r,i]

Sharding: model-parallel over R (64 r-blocks -> 8 per core); every core
reads the full x_t, weights/h/out are sliced by r; no communication.
The forward output uses only the HARD permutation (straight-through),
and a positive per-column scale cannot change a column argmax, so the
final col-normalize of sinkhorn is skipped.

Engine split (pairs of 128-row batch tiles are fused into single ops
to halve Vector-engine instruction overhead): PE does the three matmuls
(fp32 for exact argmax fidelity); ACT does exp-eviction straight out of
PSUM (fused exp(2z)) plus the exps of the sigmoid/tanh path, which is
rewritten in exps so only one ACT table set is ever loaded; DVE does
the sinkhorn reduces/scales (its ~22 full-tile 1x fp32 passes are the
critical path; GpSimd offload was tried and crashes this environment's
runtime, and no fused multiply+segmented-reduce op exists).

Measured dead ends (all on HW, deterministic key(0) inputs; gate 2e-2):
 - DVE passes run at exactly 1 elem/cycle/partition @0.96GHz (34.1us
   per merged-tile-sweep; calibrated via an iters=3 ablation, whose
   rel_err 0.0874 also matched the offline numpy sim exactly).
 - Fewer sinkhorn iterations: 4 iters -> rel_err 0.041 FAIL.
 - bf16 X: 0.079 FAIL; fp16 X: 0.031 FAIL (argmax margins are dense
   near zero, so 16-bit DVE fast modes are unusable).
 - float32r matmuls: rel_err 1.87e-2 (razor-thin vs the 2e-2 gate ->
   fp32r is tf32-class) and no measured speedup. Reverted.
 - Reciprocals on ACT (ln/exp): rel_err fine but 1134us vs 669us --
   cross-engine handoffs inside the serial sinkhorn chain cost ~us
   each. Reciprocals must stay on DVE.
 - kt_rs (PE-transpose K + 0/1-mask matmuls for the iter-1 row sums):
   917us. hp_pe (same trick for the final h-gather sums, consumer
   deferred one tile): 1234us. PE fp32 transpose+LoadStationary costs
   ~25us/tile to save 4.3us/tile of DVE, and in-order engine queues
   head-of-line block the next tile's matmuls. Both off by default.
 - tensor_tensor_scan / tensor_tensor_reduce / scalar_tensor_tensor
   accum variants: no segmented accumulation ([P,1] only) and the scan
   instruction has no fp32 fast mode, so no fusion path exists.
The flags for all of these remain in _build() for re-testing.
"""

from contextlib import ExitStack

import numpy as np

import concourse.bass as bass
import concourse.tile as tile
from concourse import bacc, mybir
from concourse.bass_utils import run_bass_kernel_spmd

N_CORES = 8
BATCH = 2048
D = 1024
R = 64
B = 16
TAU = 0.5
ITERS = 5

RG = R // N_CORES           # r-blocks per core = 8
NCOL = RG * B * B           # perm-logit cols per core = 2048
DCOL = RG * B               # diag/alpha cols per core = 128
P = 128                     # partitions
NT = BATCH // P             # batch tiles = 16
KT = D // P                 # contraction tiles = 8
F32 = mybir.dt.float32
AF = mybir.ActivationFunctionType
OP = mybir.AluOpType

# The ACT table-set chooser maps Exp -> exp_and_others and Ln ->
# natural_log (first set containing each func), which thrashes a ~2.7us
# table load on every exp<->ln switch. Our kernel only uses Exp and Ln;
# make natural_log_exp_and_others (which has both) the only candidate.
# Set ids are positional, so the dict keeps its original order/size.
import concourse.bacc as _bacc_mod
from concourse import hw_specs as _hw_specs

_orig_get_act_tables = _hw_specs.get_activation_tables
_ACT_TABLE_MODE = "ln_exp"


def _patched_get_act_tables(module_arch):
    tabs = _orig_get_act_tables(module_arch)
    if _ACT_TABLE_MODE == "exp_tanh":
        # exp_and_others holds both Exp and Tanh: one table set serves the
        # evictions and the sigmoid/tanh path with zero table swaps.
        return {
            name: (funcs if name == "exp_and_others"
                   else funcs - {AF.Exp, AF.Tanh})
            for name, funcs in tabs.items()
        }
    return {
        name: (funcs if name == "natural_log_exp_and_others"
               else funcs - {AF.Exp, AF.Ln})
        for name, funcs in tabs.items()
    }


_bacc_mod.get_activation_tables = _patched_get_act_tables


def _build(reps=1, ablate=(), kbufs=3, sbufs=3, recip_eng='dve', tpg=2, xbufs=3,
           mm_f32r=False, iters=ITERS, kt_rs=False, hp_pe=False, dv_act=False):
    ablate = set(ablate)
    global _ACT_TABLE_MODE
    _ACT_TABLE_MODE = "exp_tanh" if dv_act else "ln_exp"
    F32R = mybir.dt.float32r
    MMDT = F32R if mm_f32r else F32

    def mmcast(ap):
        return ap

    def dmacast(ap):
        return ap.bitcast(F32R) if mm_f32r else ap
    nc = bacc.Bacc("TRN2", target_bir_lowering=False, debug=False,
                   num_devices=N_CORES)
    xT = nc.dram_tensor("xT", [D, BATCH], F32, kind="ExternalInput")
    wp = nc.dram_tensor("wp", [D, NCOL], F32, kind="ExternalInput")
    wda = nc.dram_tensor("wda", [D, 2 * DCOL], F32, kind="ExternalInput")
    hs = nc.dram_tensor("hs", [BATCH, DCOL], F32, kind="ExternalInput")
    out = nc.dram_tensor("out", [BATCH, DCOL], F32, kind="ExternalOutput")

    with tile.TileContext(nc) as tc, ExitStack() as ctx:
        singles = ctx.enter_context(tc.tile_pool(name="singles", bufs=1))
        kpool = ctx.enter_context(tc.tile_pool(name="kpool", bufs=kbufs))
        small = ctx.enter_context(tc.tile_pool(name="small", bufs=sbufs))
        pspool = ctx.enter_context(tc.tile_pool(name="ps", bufs=2, space="PSUM"))
        if kt_rs:
            # psd moves to its own single buffer so psT/psRS fit in PSUM.
            # With hp_pe the perm eviction chunks shrink to 512 so psK only
            # needs 2 banks: psK 2x1 + psd 1x2 + psT/psRS/psT2/psHP 1x1 each.
            psdpool = ctx.enter_context(
                tc.tile_pool(name="psd", bufs=1, space="PSUM"))
            pstpool = ctx.enter_context(
                tc.tile_pool(name="pst", bufs=1, space="PSUM"))
            psrpool = ctx.enter_context(
                tc.tile_pool(name="psr", bufs=1, space="PSUM"))
            ktpool = ctx.enter_context(tc.tile_pool(name="ktpool", bufs=2))
        else:
            psdpool = pspool
        kchunk = 512 if hp_pe else 1024

        # Resident operands: W_perm slice, [W_diag | W_alpha] slice.
        # Load the first 512-column chunk of every k first so the first
        # tile's matmuls can start while the rest streams in.
        wps, wdas = [], []
        for k in range(KT):
            w = singles.tile([P, NCOL], MMDT, tag=f"wp{k}")
            wps.append(w)
            w2 = singles.tile([P, 2 * DCOL], MMDT, tag=f"wda{k}")
            wdas.append(w2)
        for k in range(KT):
            nc.sync.dma_start(out=wps[k][:, 0:512],
                              in_=dmacast(wp.ap()[k * P:(k + 1) * P, 0:512]))
        # bulk weight streaming rides a different DMA queue (ScalarE's)
        # so the first tile's x/h loads on SyncE's queue aren't stuck
        # behind it.
        for k in range(KT):
            nc.scalar.dma_start(out=wdas[k][:],
                                in_=dmacast(wda.ap()[k * P:(k + 1) * P, :]))
        for k in range(KT):
            nc.scalar.dma_start(out=wps[k][:, 512:NCOL],
                                in_=dmacast(wp.ap()[k * P:(k + 1) * P, 512:NCOL]))
        xpool = ctx.enter_context(tc.tile_pool(name="xpool", bufs=xbufs))

        if kt_rs:
            # identity for PE transposes; maskI8[p, m] = 1 iff p//16 == m,
            # turning a transposed K block [(i%8, j) x batch] into the 8
            # per-(g,i) row sums via one matmul per block.
            ident_np = np.eye(P, dtype=np.float32)
            mi8_np = (np.arange(P)[:, None] // 16
                      == np.arange(8)[None, :]).astype(np.float32)
            ident_dram = nc.inline_tensor(ident_np, name="identc")
            mi8_dram = nc.inline_tensor(mi8_np, name="maski8c")
            # hp-variant of the mask; absorbs the 1/2 from the tanh-based
            # sigmoid identity when dv_act is on.
            mh_dram = nc.inline_tensor(
                mi8_np * (0.5 if dv_act else 1.0), name="maskhc")
            ident = singles.tile([P, P], F32, tag="ident")
            nc.sync.dma_start(out=ident[:], in_=ident_dram.ap())
            maskI8 = singles.tile([P, 8], F32, tag="maskI8")
            nc.sync.dma_start(out=maskI8[:], in_=mi8_dram.ap())
            maskH = singles.tile([P, 8], F32, tag="maskH")
            nc.sync.dma_start(out=maskH[:], in_=mh_dram.ap())

        def act_recip(dst, src):
            if recip_eng == 'dve':
                nc.vector.reciprocal(out=dst, in_=src)
                return
            # 1/x = exp(-ln x); ln+exp share one ACT table set.
            n = src.shape[-1] if hasattr(src, 'shape') else DCe
            tmp = small.tile([P, n], F32, tag="lntmp")
            nc.scalar.activation(out=tmp, in_=src, func=AF.Ln)
            nc.scalar.activation(out=dst, in_=tmp, func=AF.Exp, scale=-1.0)

        RGe = RG * tpg          # merged r-groups across tpg batch subtiles
        DCe = DCOL * tpg
        assert not (hp_pe and not kt_rs), "hp_pe requires kt_rs machinery"
        pending = None
        for bt in range((NT // tpg) * reps):
            bt = bt % (NT // tpg)

            # per-subtile x^T slices, streamed
            xts = []
            for s_ in range(tpg):
                xsub = []
                for k in range(KT):
                    xt = xpool.tile([P, P], MMDT, tag=f"xt{k}_{s_}")
                    nc.sync.dma_start(
                        out=xt,
                        in_=dmacast(
                            xT.ap()[k * P:(k + 1) * P,
                                    (bt * tpg + s_) * P:(bt * tpg + s_ + 1) * P]))
                    xsub.append(xt)
                xts.append(xsub)

            K_t = kpool.tile([P, RGe, B, B], F32, tag="K")
            Kflat = K_t[:].rearrange("p g i j -> p (g i j)")

            # logits matmul in PSUM chunks; evict through ACT with fused
            # exp(2*z)  [1/TAU = 2].
            for s_ in range(tpg):
                for ch in range(NCOL // kchunk):
                    ps = pspool.tile([P, kchunk], F32, tag="psK")
                    for nb in range(kchunk // 512):
                        ncol0 = ch * kchunk + nb * 512
                        for k in range(KT):
                            nc.tensor.matmul(
                                out=ps[:, nb * 512:(nb + 1) * 512],
                                lhsT=mmcast(xts[s_][k][:]),
                                rhs=mmcast(wps[k][:, ncol0:ncol0 + 512]),
                                start=(k == 0),
                                stop=(k == KT - 1),
                            )
                    nc.scalar.activation(
                            out=Kflat[:, s_ * NCOL + ch * kchunk:
                                      s_ * NCOL + (ch + 1) * kchunk],
                            in_=ps[:],
                            func=AF.Exp,
                            scale=2.0,
                        )

            # K^T via PE transpose, then per-block mask matmuls give the
            # iteration-1 row sums in PSUM without a DVE reduce pass.
            psRS = None
            if kt_rs:
                nblk = RGe * 2
                psRS = psrpool.tile([P, RGe * B], F32, tag="psRS")
                for w in range(nblk // 4):
                    psT = pstpool.tile([P, 512], F32, tag="psT")
                    for q in range(4):
                        c = w * 4 + q
                        nc.tensor.transpose(
                            out=psT[:, q * P:(q + 1) * P],
                            in_=Kflat[:, c * P:(c + 1) * P],
                            identity=ident[:])
                    ktb = ktpool.tile([P, 512], F32, tag="ktb")
                    nc.scalar.copy(out=ktb[:], in_=psT[:])
                    for q in range(4):
                        c = w * 4 + q
                        g, h = c // 2, c % 2
                        nc.tensor.matmul(
                            out=psRS[:, g * B + 8 * h:g * B + 8 * h + 8],
                            lhsT=ktb[:, q * P:(q + 1) * P],
                            rhs=maskI8[:],
                            start=True, stop=True)

            # diag/alpha matmul: [x @ Wd | x @ Wa] -> one PSUM bank.
            psd = psdpool.tile([P, tpg * 2 * DCOL], F32, tag="psD")
            for s_ in range(tpg):
                for k in range(KT):
                    nc.tensor.matmul(
                        out=psd[:, s_ * 2 * DCOL:(s_ + 1) * 2 * DCOL],
                        lhsT=mmcast(xts[s_][k][:]),
                        rhs=mmcast(wdas[k][:]),
                        start=(k == 0),
                        stop=(k == KT - 1),
                    )
            if dv_act:
                # sigmoid(a)*tanh(d) = (1 + tanh(a/2))*tanh(d) / 2; both
                # tanhs share the exp table set so ACT never swaps tables.
                # dv holds 2x the true value; the 1/2 is folded into the
                # h-mask multiply below.
                t2 = small.tile([P, DCe], F32, tag="e2d")
                t1 = small.tile([P, DCe], F32, tag="ena")
                for s_ in range(tpg):
                    nc.scalar.activation(
                        out=t2[:, s_ * DCOL:(s_ + 1) * DCOL],
                        in_=psd[:, s_ * 2 * DCOL:s_ * 2 * DCOL + DCOL],
                        func=AF.Tanh)
                    nc.scalar.activation(
                        out=t1[:, s_ * DCOL:(s_ + 1) * DCOL],
                        in_=psd[:, s_ * 2 * DCOL + DCOL:(s_ + 1) * 2 * DCOL],
                        func=AF.Tanh, scale=0.5)
                dv = small.tile([P, DCe], F32, tag="dv")
                nc.vector.scalar_tensor_tensor(out=dv, in0=t1, scalar=1.0,
                                               in1=t2, op0=OP.add, op1=OP.mult)
            else:
                # sigmoid(a)*tanh(d) = (e2d - 1) / ((1 + e2d) * (1 + ena))
                e2d = small.tile([P, DCe], F32, tag="e2d")
                ena = small.tile([P, DCe], F32, tag="ena")
                for s_ in range(tpg):
                    nc.scalar.activation(
                        out=e2d[:, s_ * DCOL:(s_ + 1) * DCOL],
                        in_=psd[:, s_ * 2 * DCOL:s_ * 2 * DCOL + DCOL],
                        func=AF.Exp, scale=2.0)
                    nc.scalar.activation(
                        out=ena[:, s_ * DCOL:(s_ + 1) * DCOL],
                        in_=psd[:, s_ * 2 * DCOL + DCOL:(s_ + 1) * 2 * DCOL],
                        func=AF.Exp, scale=-1.0)
                num = small.tile([P, DCe], F32, tag="num")
                nc.vector.tensor_scalar_sub(out=num, in0=e2d, scalar1=1.0)
                den = small.tile([P, DCe], F32, tag="den")
                nc.vector.scalar_tensor_tensor(out=den, in0=e2d, scalar=1.0,
                                               in1=ena, op0=OP.add, op1=OP.mult)
                dpa = small.tile([P, DCe], F32, tag="dpa")
                # denom = (1+e2d)*(1+ena) = (e2d+1) + (e2d+1)*ena
                nc.vector.scalar_tensor_tensor(out=dpa, in0=e2d, scalar=1.0,
                                               in1=den, op0=OP.add, op1=OP.add)
                rden = small.tile([P, DCe], F32, tag="rden")
                act_recip(rden, dpa)
                dv = small.tile([P, DCe], F32, tag="dv")
                nc.vector.tensor_mul(out=dv, in0=num, in1=rden)

            def sinkhorn_final(g0, ng, after_it0=None):
                # sinkhorn + hard-permutation + output for r-groups
                # [g0, g0+ng) of this tile's merged K. Splitting the first
                # tile into halves lets DVE start before all evictions land.
                Xs = K_t[:, g0:g0 + ng]                 # [P, ng, i, j]
                Xti = Xs.transpose([0, 1, 3, 2])        # [P, ng, j, i]
                DCs = ng * B
                csl = slice(g0 * B, (g0 + ng) * B)

                def bcast_gi(t):   # (g,i)-indexed -> broadcast over j
                    return (t[:].rearrange("p (g i) -> p g i", g=ng)
                            .unsqueeze(3).to_broadcast([P, ng, B, B]))

                def bcast_gj(t):   # (g,j)-indexed -> broadcast over i
                    return (t[:].rearrange("p (g j) -> p g j", g=ng)
                            .unsqueeze(2).to_broadcast([P, ng, B, B]))

                for it in range(iters):
                    if it == 0 and psRS is not None:
                        rs = psRS[:, csl]
                    else:
                        rs = small.tile([P, DCs], F32, tag="rs")
                        nc.vector.reduce_sum(out=rs, in_=Xs,
                                             axis=mybir.AxisListType.X)
                    rr = small.tile([P, DCs], F32, tag="rr")
                    act_recip(rr, rs)
                    nc.vector.tensor_tensor(out=Xs, in0=Xs, in1=bcast_gi(rr),
                                            op=OP.mult)
                    if it == 0 and after_it0 is not None:
                        after_it0()
                    if it < iters - 1:
                        cs = small.tile([P, DCs], F32, tag="cs")
                        nc.vector.reduce_sum(out=cs, in_=Xti,
                                             axis=mybir.AxisListType.X)
                        rc = small.tile([P, DCs], F32, tag="rc")
                        act_recip(rc, cs)
                        nc.vector.tensor_tensor(out=Xs, in0=Xs,
                                                in1=bcast_gj(rc), op=OP.mult)

                # column max over i -> hard assignment mask -> h gather.
                if 'argmax' in ablate:
                    nc.vector.tensor_mul(out=o_t[:, csl],
                                         in0=h_t[:, csl], in1=dv[:, csl])
                    return
                M = small.tile([P, DCs], F32, tag="M")
                nc.vector.reduce_max(out=M, in_=Xti, axis=mybir.AxisListType.X)
                nc.vector.tensor_tensor(out=Xs, in0=Xs, in1=bcast_gj(M),
                                        op=OP.is_equal)
                nc.vector.tensor_tensor(out=Xs, in0=Xs,
                                        in1=bcast_gj(h_t[:, csl]), op=OP.mult)
                if hp_pe:
                    # transpose the masked-h product and mask-matmul the
                    # per-(g,i) sums straight into psHP [batch, (g,i)];
                    # the hp*dv multiply is deferred into the next tile.
                    for w in range(2 * ng // 4):
                        psT2 = pstpool.tile([P, 512], F32, tag="psT2")
                        for q in range(4):
                            c = 2 * g0 + w * 4 + q
                            nc.tensor.transpose(
                                out=psT2[:, q * P:(q + 1) * P],
                                in_=Kflat[:, c * P:(c + 1) * P],
                                identity=ident[:])
                        ktb2 = ktpool.tile([P, 512], F32, tag="ktb2")
                        nc.scalar.copy(out=ktb2[:], in_=psT2[:])
                        for q in range(4):
                            c = 2 * g0 + w * 4 + q
                            g, h_ = c // 2, c % 2
                            nc.tensor.matmul(
                                out=psHP[:, g * B + 8 * h_:g * B + 8 * h_ + 8],
                                lhsT=ktb2[:, q * P:(q + 1) * P],
                                rhs=maskH[:],
                                start=True, stop=True)
                    return
                hp = small.tile([P, DCs], F32, tag="hp")
                nc.vector.reduce_sum(out=hp, in_=Xs, axis=mybir.AxisListType.X)
                if dv_act:
                    # dv holds 2x sigmoid*tanh; halve here.
                    nc.vector.scalar_tensor_tensor(
                        out=o_t[:, csl], in0=hp, scalar=0.5, in1=dv[:, csl],
                        op0=OP.mult, op1=OP.mult)
                else:
                    nc.vector.tensor_mul(out=o_t[:, csl], in0=hp,
                                         in1=dv[:, csl])

            h_t = small.tile([P, DCe], F32, tag="h")
            for s_ in range(tpg):
                b0 = (bt * tpg + s_) * P
                nc.sync.dma_start(out=h_t[:, s_ * DCOL:(s_ + 1) * DCOL],
                                  in_=hs.ap()[b0:b0 + P, :])
            o_t = small.tile([P, DCe], F32, tag="o")
            psHP = None
            if hp_pe:
                psHP = psrpool.tile([P, RGe * B], F32, tag="psHP")

            flush_prev = None
            if pending is not None:
                o_p, dv_p, psHP_p, bt_p = pending

                def flush_prev(o_p=o_p, dv_p=dv_p, psHP_p=psHP_p, bt_p=bt_p):
                    nc.vector.tensor_mul(out=o_p[:], in0=psHP_p[:], in1=dv_p[:])
                    for s_ in range(tpg):
                        b0 = (bt_p * tpg + s_) * P
                        nc.sync.dma_start(
                            out=out.ap()[b0:b0 + P, :],
                            in_=o_p[:, s_ * DCOL:(s_ + 1) * DCOL])
                pending = None

            if bt == 0:
                q = RGe // (2 * tpg)   # one eviction's worth of r-groups
                for s_ in range(2 * tpg):
                    sinkhorn_final(s_ * q, q,
                                   after_it0=flush_prev if s_ == 0 else None)
            else:
                sinkhorn_final(0, RGe, after_it0=flush_prev)

            if hp_pe:
                pending = (o_t, dv, psHP, bt)
            else:
                for s_ in range(tpg):
                    b0 = (bt * tpg + s_) * P
                    nc.sync.dma_start(out=out.ap()[b0:b0 + P, :],
                                      in_=o_t[:, s_ * DCOL:(s_ + 1) * DCOL])
        if pending is not None:
            o_p, dv_p, psHP_p, bt_p = pending
            nc.vector.tensor_mul(out=o_p[:], in0=psHP_p[:], in1=dv_p[:])
            for s_ in range(tpg):
                b0 = (bt_p * tpg + s_) * P
                nc.sync.dma_start(out=out.ap()[b0:b0 + P, :],
                                  in_=o_p[:, s_ * DCOL:(s_ + 1) * DCOL])

    nc.compile()
    return nc


_NC = None


def _get_nc():
    global _NC
    if _NC is None:
        _NC = _build()
    return _NC


def kernel(x_t, h, W_perm, W_diag, W_alpha):
    x_t = np.ascontiguousarray(np.asarray(x_t, dtype=np.float32))
    h = np.asarray(h, dtype=np.float32)
    W_perm = np.asarray(W_perm, dtype=np.float32)
    W_diag = np.asarray(W_diag, dtype=np.float32)
    W_alpha = np.asarray(W_alpha, dtype=np.float32)

    xT = np.ascontiguousarray(x_t.T)                          # [D, BATCH]
    wp4 = W_perm.reshape(D, R, B * B)
    wd3 = W_diag.reshape(D, R, B)
    wa3 = W_alpha.reshape(D, R, B)
    h3 = h.reshape(BATCH, R, B)

    in_maps = []
    for c in range(N_CORES):
        rsl = slice(c * RG, (c + 1) * RG)
        in_maps.append({
            "xT": xT,
            "wp": np.ascontiguousarray(wp4[:, rsl].reshape(D, NCOL)),
            "wda": np.ascontiguousarray(
                np.concatenate([wd3[:, rsl].reshape(D, DCOL),
                                wa3[:, rsl].reshape(D, DCOL)], axis=1)),
            "hs": np.ascontiguousarray(h3[:, rsl].reshape(BATCH, DCOL)),
        })

    global _last_in_maps
    _last_in_maps = in_maps
    res = run_bass_kernel_spmd(_get_nc(), in_maps, core_ids=list(range(N_CORES)))
    parts = [res.results[c]["out"].reshape(BATCH, RG, B) for c in range(N_CORES)]
    return np.concatenate(parts, axis=1).reshape(BATCH, R * B).astype(np.float32)

